# revision 15
# baseline (speedup 1.0000x reference)
import sys

sys.path.insert(0, "/opt/trn_rl_repo")

import hashlib

import numpy as np
import ml_dtypes

import concourse.bass as bass
import concourse.mybir as mybir
import concourse.tile as tile
from concourse.library_config import mlp
from concourse.masks import make_identity
from concourse.vector_clock import ScopedClock

dt = mybir.dt
AF = mybir.ActivationFunctionType
ALU = mybir.AluOpType
bf16 = ml_dtypes.bfloat16

N_NODES = 100000
F_IN = 128
N_CLASSES = 40
NCORES = 8
NSH = 12500
NT = 98
NSHP = NT * 128  # 12544
NJ = 4  # source row-slices (separate AllGather + gather table per slice)
SLICE_T = [0, 25, 50, 74, 98]  # tile boundaries of the slices
ROW_START = [t * 128 for t in SLICE_T]  # [0, 3200, 6400, 9472, 12544]
ROWS_J = [ROW_START[j + 1] - ROW_START[j] for j in range(NJ)]
BT = 6  # tiles per gather block (= dedicated PSUM accumulator banks)
NB = (NT + BT - 1) // BT  # 17 blocks (last one has 2 tiles)
KCAP = 8  # hard ucode limit: <=1024 indices per dma_gather call


class PatchedTileContext(tile.TileContext):
    # walrus CoreV3 codegen accepts at most 1 sem wait on most instruction
    # structs; spread the final-drain waits over 1-wait nops.
    def _drain_and_barrier(self, tick_clock, wait_clock):
        collector = self.nc.sync.nop(nofuse=True)
        wait_clock.add_sem_waits(
            collector.ins, ScopedClock({None: tick_clock.global_clock})
        )
        si = collector.ins.sync_info
        waits = list(si.on_wait) if si and si.on_wait else []
        if len(waits) > 1:
            si.on_wait = waits[:1]
            for w in waits[1:]:
                extra = self.nc.sync.nop(nofuse=True)
                extra.ins.sync_info = mybir.SyncInfo(on_wait=[w], on_update=[])
        self.nc.sync.drain()
        self.nc.all_engine_barrier()
        assert self.sems is not None
        popped = self.nc._tile_sem_poison_stack.pop()
        assert popped is self._sem_poison
        self.nc.clear_and_free_semaphores(list(self.sems.allocated().values()))
        self.nc.all_engine_barrier()


def _split_excess_waits(nc, max_waits=1):
    # Same walrus limit for ordinary instructions: move excess sem waits onto
    # single-wait carrier instructions on the same engine, inserted just
    # before (per-engine order makes the stall equivalent).
    cnt = 0
    for f in nc.m.functions:
        for bb in f.blocks:
            insns = bb.instructions
            i = 0
            while i < len(insns):
                ins = insns[i]
                si = getattr(ins, "sync_info", None)
                waits = list(si.on_wait) if si is not None and si.on_wait else []
                if len(waits) > max_waits:
                    si.on_wait = waits[:1]
                    for w in waits[1:]:
                        if ins.engine == mybir.EngineType.Pool:
                            nop = mybir.InstEventSemaphore(
                                name=f"waitsplit_{cnt}", ins=[], outs=[]
                            )
                        else:
                            nop = mybir.InstNoOp(
                                name=f"waitsplit_{cnt}", ins=[], outs=[]
                            )
                        cnt += 1
                        nop.engine = ins.engine
                        nop.sync_info = mybir.SyncInfo(on_wait=[w], on_update=[])
                        insns.insert(i, nop)
                        i += 1
                i += 1
    return cnt


def _preprocess(edge_index):
    # Buckets: (block of BT dst tiles, source row-slice j, tile in block).
    # Chunks of one (block, j) pair are contiguous, so they gather from
    # table_j with a few large dma_gather calls (<= KCAP*128 idxs each, the
    # ucode limit), and the j-sliced tables let the per-slice AllGathers
    # overlap with compute.
    src = np.asarray(edge_index[0], dtype=np.int64)
    dst = np.asarray(edge_index[1], dtype=np.int64)
    deg = np.bincount(dst, minlength=N_NODES).astype(np.float32) + 1.0
    dinv = (1.0 / np.sqrt(deg)).astype(np.float32)

    NKEY = NB * NJ * BT
    core_of = dst // NSH
    per_core = []
    counts = np.zeros((NCORES, NKEY), np.int64)
    row_start = np.asarray(ROW_START[1:NJ], dtype=np.int64)  # [3200, 6400, 9472]
    for c in range(NCORES):
        m = core_of == c
        es = src[m]
        ed = dst[m] - c * NSH
        t = ed >> 7
        slot = ed & 127
        sc = es // NSH
        r = es % NSH
        j = np.searchsorted(row_start, r, side="right")
        lidx = sc * np.asarray(ROWS_J)[j] + (r - np.asarray(ROW_START)[j])
        key = (((t // BT) * NJ) + j) * BT + (t % BT)
        order = np.argsort(key, kind="stable")
        key = key[order]
        lidx = lidx[order]
        slot = slot[order]
        cnt = np.bincount(key, minlength=NKEY)
        counts[c] = cnt
        per_core.append((key, lidx, slot, cnt))

    K = np.ceil(counts / 128.0).astype(np.int64).max(axis=0)  # [NKEY]
    nchunk = int(K.sum())
    tot = nchunk * 128
    chunk_off = np.concatenate([[0], np.cumsum(K)]).astype(np.int64)

    idx_ws, dst_ws, dinv_cs = [], [], []
    for c in range(NCORES):
        key, lidx, slot, cnt = per_core[c]
        starts = np.cumsum(cnt) - cnt
        jj = np.arange(len(key)) - starts[key]
        pos = chunk_off[key] * 128 + jj
        idx_flat = np.zeros(tot, np.int16)
        dst_flat = np.full(tot, 999.0, np.float32)
        idx_flat[pos] = lidx.astype(np.int16)
        dst_flat[pos] = slot.astype(np.float32)
        # [128, tot//16]: 16-row wrap replicated 8x (one copy per gpsimd core)
        idx_ws.append(
            np.ascontiguousarray(np.tile(idx_flat.reshape(tot // 16, 16).T, (8, 1)))
        )
        dst_ws.append(
            np.ascontiguousarray(dst_flat.reshape(nchunk, 128).T).astype(bf16)
        )
        dloc = np.zeros(NSHP, np.float32)
        dloc[:NSH] = dinv[c * NSH : (c + 1) * NSH]
        dinv_cs.append(np.ascontiguousarray(dloc.reshape(NT, 128).T))

    kbj = K.reshape(NB * NJ, BT).sum(axis=1)
    nmaxbj = int(kbj.max())
    iota = np.tile(
        np.arange(128, dtype=np.float32)[None, :], (128, nmaxbj)
    ).reshape(128, nmaxbj, 128)
    return dict(
        K=K,
        nchunk=nchunk,
        tot=tot,
        chunk_off=chunk_off,
        nmaxbj=nmaxbj,
        idx_ws=idx_ws,
        dst_ws=dst_ws,
        dinv_cs=dinv_cs,
        iota=np.ascontiguousarray(iota).astype(bf16),
    )


def _build(meta, ablate=()):
    K = meta["K"]
    nchunk = meta["nchunk"]
    tot = meta["tot"]
    chunk_off = meta["chunk_off"]
    nmaxbj = meta["nmaxbj"]

    nc = bass.Bass(num_devices=NCORES, num_swdge_queues=4)
    xT_in = nc.dram_tensor("xT_s", [F_IN, NSHP], dt.bfloat16, kind="ExternalInput")
    w1_in = nc.dram_tensor("w1", [F_IN, F_IN], dt.bfloat16, kind="ExternalInput")
    w2_in = nc.dram_tensor("w2", [F_IN, 128], dt.bfloat16, kind="ExternalInput")
    dinv_in = nc.dram_tensor("dinv_c", [128, NT], dt.float32, kind="ExternalInput")
    idx_in = nc.dram_tensor("idx_w", [128, tot // 16], dt.int16, kind="ExternalInput")
    dst_in = nc.dram_tensor("dst_w", [128, nchunk], dt.bfloat16, kind="ExternalInput")
    iota_in = nc.dram_tensor(
        "iota_r", [128, nmaxbj, 128], dt.bfloat16, kind="ExternalInput"
    )
    out_t = nc.dram_tensor("out_s", [NSHP, N_CLASSES], dt.float32, kind="ExternalOutput")

    with PatchedTileContext(nc) as tc:
        with (
            tc.tile_pool(name="sbuf", bufs=1) as pool,
            tc.tile_pool(name="psum", bufs=1, space="PSUM") as psum,
            tc.tile_pool(name="dram", bufs=1, space="DRAM") as dram,
        ):
            xT_full = pool.tile([F_IN, NSHP], dt.bfloat16)
            w1_t = pool.tile([F_IN, F_IN], dt.bfloat16)
            w2_t = pool.tile([F_IN, 128], dt.bfloat16)
            dinv_t = pool.tile([128, NT], dt.float32)
            idx_t = pool.tile([128, tot // 16], dt.int16)
            dst_t = pool.tile([128, nchunk], dt.bfloat16)
            iota_t = pool.tile([128, nmaxbj, 128], dt.bfloat16)
            ident = pool.tile([128, 128], dt.float32)
            h_all = pool.tile([128, NT, F_IN], dt.bfloat16)
            z_all = pool.tile([128, NT, N_CLASSES], dt.float32)
            zs_all = pool.tile([128, NT, N_CLASSES], dt.float32)
            znorm = pool.tile([128, NT, N_CLASSES], dt.float32)
            nc.gpsimd.load_library(mlp)
            for d_ap, s_ap in [
                (xT_full, xT_in),
                (w1_t, w1_in),
                (w2_t, w2_in),
                (dinv_t, dinv_in),
                (idx_t, idx_in),
                (dst_t, dst_in),
                (iota_t, iota_in),
            ]:
                nc.sync.dma_start(d_ap[:], s_ap[:])
            make_identity(nc, ident[:])

            bounce1 = [
                dram.tile([ROWS_J[j], F_IN], dt.bfloat16, name=f"bounce1_{j}")
                for j in range(NJ)
            ]
            table1 = [
                dram.tile(
                    [NCORES * ROWS_J[j], F_IN],
                    dt.bfloat16,
                    addr_space="Shared",
                    name=f"table1_{j}",
                )
                for j in range(NJ)
            ]
            bounce2 = [
                dram.tile([ROWS_J[j], 128], dt.bfloat16, name=f"bounce2_{j}")
                for j in range(NJ)
            ]
            table2 = [
                dram.tile(
                    [NCORES * ROWS_J[j], 128],
                    dt.bfloat16,
                    addr_space="Shared",
                    name=f"table2_{j}",
                )
                for j in range(NJ)
            ]
            b1_views = [
                bounce1[j][:].rearrange("(t p) f -> p t f", p=128) for j in range(NJ)
            ]
            b2_views = [
                bounce2[j][:].rearrange("(t p) f -> p t f", p=128) for j in range(NJ)
            ]
            out_view = out_t[:].rearrange("(t p) c -> p t c", p=128)

            def all_gather(bounce, table):
                nc.gpsimd.collective_compute(
                    "AllGather",
                    ALU.bypass,
                    replica_groups=[list(range(NCORES))],
                    ins=[bounce.opt()],
                    outs=[table.opt()],
                )

            # Phase A: h~ = dinv * (x @ W1) in bf16; publish + AllGather each
            # row-slice as soon as it is complete.
            _sidA, _ = nc.enter_named_scope("phaseA", False)
            for j in range(NJ):
                t0, t1 = SLICE_T[j], SLICE_T[j + 1]
                for t in range(t0, t1):
                    mm = psum.tile([128, F_IN], dt.float32, name="mm", bufs=1)
                    nc.tensor.matmul(
                        mm[:],
                        lhsT=xT_full[:, t * 128 : (t + 1) * 128],
                        rhs=w1_t[:],
                        start=True,
                        stop=True,
                    )
                    nc.scalar.activation(
                        h_all[:, t, :],
                        mm[:],
                        AF.Copy,
                        bias=0.0,
                        scale=dinv_t[:, t : t + 1],
                    )
                nc.sync.dma_start(
                    b1_views[j][:, :, :], h_all[:, t0:t1, :]
                )
                if "cc" not in ablate:
                    all_gather(bounce1[j], table1[j])
            nc.leave_named_scope("phaseA", _sidA, False)

            # gpsimd registers are scarce: one per distinct idx count, reused
            reg_cache = {}

            def nreg(v):
                if v not in reg_cache:
                    reg_cache[v] = nc.gpsimd.to_reg(v)
                return reg_cache[v]

            def agg_layer(scope, tables, elem, tail):
                # A start=True matmul resets the whole PSUM bank, so each tile
                # gets a DEDICATED bank for its whole accumulation group (the
                # group spans the NJ slice passes; groups of different tiles
                # interleave only across banks, never within one).  BT=6 acc
                # banks + 1 mm + 1 tp = 8.
                _sid, _ = nc.enter_named_scope(scope, False)
                for b in range(NB):
                    nbt = min(BT, NT - b * BT)
                    accs = [
                        psum.tile([128, 128], dt.float32, name="acc", bufs=BT)
                        for _ in range(nbt)
                    ]
                    total = [
                        int(sum(K[(b * NJ + j) * BT + tp] for j in range(NJ)))
                        for tp in range(nbt)
                    ]
                    done = [0] * nbt
                    if "pe" in ablate:
                        for a in accs:
                            nc.vector.memset(a[:], 0.0)
                    for j in range(NJ):
                        k0 = (b * NJ + j) * BT
                        off = int(chunk_off[k0])
                        n = int(chunk_off[k0 + BT]) - off
                        if n == 0:
                            continue
                        # one SBUF tile per dma_gather call (<= KCAP chunks):
                        # small tiles with a deep pool let the in-order Pool
                        # engine issue many calls ahead of PE/DVE consumption
                        gcalls = []
                        for p0 in range(0, n, KCAP):
                            kp = min(KCAP, n - p0)
                            o = off + p0
                            g = pool.tile(
                                [128, KCAP, elem], dt.bfloat16, name="g", bufs=10
                            )
                            if "gather" in ablate and "pe" not in ablate:
                                nc.scalar.memset(g[:], 0.0)
                            if "gather" not in ablate:
                                nc.gpsimd.dma_gather(
                                    g[:, 0:kp, :],
                                    tables[j][:],
                                    idx_t[:, o * 8 : (o + kp) * 8],
                                    num_idxs=kp * 128,
                                    num_idxs_reg=nreg(kp * 128),
                                    elem_size=elem,
                                    queue_num=j,
                                )
                            gcalls.append(g)
                        sel = pool.tile(
                            [128, nmaxbj, 128], dt.bfloat16, name="sel", bufs=2
                        )
                        nc.vector.tensor_tensor(
                            out=sel[:, 0:n, :],
                            in0=dst_t[:, off : off + n].to_broadcast([128, n, 128]),
                            in1=iota_t[:, 0:n, :],
                            op=ALU.is_equal,
                        )
                        pos = 0
                        for tp in range(nbt):
                            kk = int(K[k0 + tp])
                            if "pe" in ablate:
                                pos += kk
                                continue
                            for _ in range(kk):
                                nc.tensor.matmul(
                                    accs[tp][:],
                                    lhsT=sel[:, pos, :],
                                    rhs=gcalls[pos // KCAP][:, pos % KCAP, :],
                                    start=(done[tp] == 0),
                                    stop=(done[tp] == total[tp] - 1),
                                )
                                done[tp] += 1
                                pos += 1
                    for tp in range(nbt):
                        tail(b * BT + tp, accs[tp][:])
                nc.leave_named_scope(scope, _sid, False)

            # Phase B tail: relu(dinv*(acc + h~)), then layer-2 transform.
            # z slices are published + AllGathered as they complete.
            z16_state = {}

            def tail_b(t, acc):
                agg = pool.tile([128, 128], dt.float32, name="agg", bufs=2)
                nc.vector.tensor_tensor(
                    out=agg[:], in0=acc, in1=h_all[:, t, :], op=ALU.add
                )
                h1 = pool.tile([128, 128], dt.float32, name="h1", bufs=2)
                nc.scalar.activation(
                    h1[:], agg[:], AF.Relu, bias=0.0, scale=dinv_t[:, t : t + 1]
                )
                tp_ = psum.tile([128, 128], dt.float32, name="tp", bufs=1)
                nc.tensor.transpose(tp_[:], h1[:], ident[:])
                hT = pool.tile([128, 128], dt.bfloat16, name="hT", bufs=2)
                nc.scalar.copy(hT[:], tp_[:])
                mm = psum.tile([128, 128], dt.float32, name="mm", bufs=1)
                nc.tensor.matmul(mm[:], lhsT=hT[:], rhs=w2_t[:], start=True, stop=True)
                nc.scalar.activation(
                    z_all[:, t, :],
                    mm[:, 0:N_CLASSES],
                    AF.Copy,
                    bias=0.0,
                    scale=dinv_t[:, t : t + 1],
                )
                j = next(jj for jj in range(NJ) if SLICE_T[jj] <= t < SLICE_T[jj + 1])
                t0, t1 = SLICE_T[j], SLICE_T[j + 1]
                if t == t0:
                    z16_state["tile"] = pool.tile(
                        [128, 25, 128], dt.bfloat16, name="z16", bufs=2
                    )
                z16 = z16_state["tile"]
                nc.scalar.activation(
                    z16[:, t - t0, :],
                    mm[:],
                    AF.Copy,
                    bias=0.0,
                    scale=dinv_t[:, t : t + 1],
                )
                if t == t1 - 1:
                    nc.sync.dma_start(
                        b2_views[j][:, :, :], z16[:, 0 : t1 - t0, :]
                    )
                    if "cc" not in ablate:
                        all_gather(bounce2[j], table2[j])

            agg_layer("phaseB", table1, F_IN, tail_b)

            # Phase C tail: z_sum = acc + z
            def tail_c(t, acc):
                nc.vector.tensor_tensor(
                    out=zs_all[:, t, :],
                    in0=acc[:, 0:N_CLASSES],
                    in1=z_all[:, t, :],
                    op=ALU.add,
                )

            agg_layer("phaseC", table2, 128, tail_c)

            # batched log_softmax, write out
            _sidS, _ = nc.enter_named_scope("softmax", False)
            nc.vector.tensor_tensor(
                out=znorm[:],
                in0=zs_all[:],
                in1=dinv_t[:, :].to_broadcast([128, NT, N_CLASSES]),
                op=ALU.mult,
            )
            mx = pool.tile([128, NT, 1], dt.float32, name="mx")
            nc.vector.tensor_reduce(mx[:], znorm[:], mybir.AxisListType.X, ALU.max)
            nc.vector.tensor_tensor(
                out=zs_all[:],
                in0=znorm[:],
                in1=mx[:, :, 0].to_broadcast([128, NT, N_CLASSES]),
                op=ALU.subtract,
            )
            nc.scalar.activation(znorm[:], zs_all[:], AF.Exp, bias=0.0, scale=1.0)
            sm = pool.tile([128, NT, 1], dt.float32, name="sm")
            nc.vector.tensor_reduce(sm[:], znorm[:], mybir.AxisListType.X, ALU.add)
            ls = pool.tile([128, NT, 1], dt.float32, name="ls")
            nc.scalar.activation(ls[:], sm[:], AF.Ln, bias=0.0, scale=1.0)
            nc.vector.tensor_tensor(
                out=znorm[:],
                in0=zs_all[:],
                in1=ls[:, :, 0].to_broadcast([128, NT, N_CLASSES]),
                op=ALU.subtract,
            )
            nc.sync.dma_start(out_view[:, :, :], znorm[:])
            nc.leave_named_scope("softmax", _sidS, False)

    _split_excess_waits(nc)
    mybir.codegen_inst_isa_subclasses(nc)
    return nc


def _make_runner(nc):
    import jax
    from jax.sharding import Mesh, PartitionSpec

    try:
        from jax.experimental.shard_map import shard_map
    except ImportError:
        from jax.shard_map import shard_map

    from concourse.bass2jax import (
        _bass_exec_p,
        install_neuronx_cc_hook,
        partition_id_tensor,
    )

    install_neuronx_cc_hook()
    assert nc.dbg_addr is None
    partition_name = nc.partition_id_tensor.name if nc.partition_id_tensor else None

    in_names, out_names, out_avals = [], [], []
    for alloc in nc.m.functions[0].allocations:
        if not isinstance(alloc, mybir.MemoryLocationSet):
            continue
        name = alloc.memorylocations[0].name
        if alloc.kind == "ExternalInput":
            if name != partition_name:
                in_names.append(name)
        elif alloc.kind == "ExternalOutput":
            out_names.append(name)
            shape = tuple(alloc.tensor_shape)
            dtype = mybir.dt.np(alloc.dtype)
            out_avals.append(jax.core.ShapedArray(shape, dtype))
    n_params = len(in_names)
    n_outs = len(out_avals)
    all_names = in_names + out_names
    if partition_name is not None:
        all_names = all_names + [partition_name]
    donate = tuple(range(n_params, n_params + n_outs))

    def _body(*args):
        operands = list(args)
        if partition_name is not None:
            operands.append(partition_id_tensor())
        outs = _bass_exec_p.bind(
            *operands,
            out_avals=tuple(out_avals),
            in_names=tuple(all_names),
            out_names=tuple(out_names),
            lowering_input_output_aliases=(),
            sim_require_finite=True,
            sim_require_nnan=True,
            nc=nc,
        )
        return tuple(outs)

    devices = jax.devices()[:NCORES]
    mesh = Mesh(np.asarray(devices), ("core",))
    in_specs = (PartitionSpec("core"),) * (n_params + n_outs)
    out_specs = (PartitionSpec("core"),) * n_outs
    sharded = jax.jit(
        shard_map(
            _body, mesh=mesh, in_specs=in_specs, out_specs=out_specs, check_rep=False
        ),
        donate_argnums=donate,
        keep_unused=True,
    )

    state = {"dev_in": None, "dev_key": None}

    def run(in_maps):
        per_core = [[np.asarray(m[name]) for name in in_names] for m in in_maps]
        concat_in = [
            np.concatenate([per_core[c][i] for c in range(NCORES)], axis=0)
            for i in range(n_params)
        ]
        hkey = hashlib.sha1()
        for a in concat_in:
            hkey.update(a.tobytes())
        hkey = hkey.hexdigest()
        if state["dev_key"] != hkey:
            from jax.sharding import NamedSharding

            state["dev_in"] = [
                jax.device_put(a, NamedSharding(mesh, PartitionSpec("core")))
                for a in concat_in
            ]
            state["dev_key"] = hkey
        concat_zeros = [
            np.zeros((NCORES * a.shape[0], *a.shape[1:]), a.dtype) for a in out_avals
        ]
        out_arrs = sharded(*state["dev_in"], *concat_zeros)
        jax.block_until_ready(out_arrs)
        return [
            [
                np.asarray(out_arrs[i]).reshape(NCORES, *out_avals[i].shape)[c]
                for i in range(n_outs)
            ]
            for c in range(NCORES)
        ]

    run.sharded = sharded
    run.state = state
    run.mesh = mesh
    run.out_avals = out_avals
    run.body = _body
    run.n_params = n_params
    return run


_CACHE = {}


def kernel(**inputs):
    x = np.asarray(inputs["x"], np.float32)
    ei = np.asarray(inputs["edge_index"])
    W1 = np.asarray(inputs["W1"], np.float32)
    W2 = np.asarray(inputs["W2"], np.float32)
    b1 = np.asarray(inputs["b1"], np.float32)
    b2 = np.asarray(inputs["b2"], np.float32)
    assert not b1.any() and not b2.any(), "nonzero biases not supported"

    key = hashlib.sha1(ei.tobytes()).hexdigest()
    st = _CACHE.get(key)
    if st is None:
        meta = _preprocess(ei)
        nc = _build(meta)
        runner = _make_runner(nc)
        st = {"meta": meta, "runner": runner}
        _CACHE.clear()
        _CACHE[key] = st
    meta = st["meta"]

    w2p = np.zeros((F_IN, 128), np.float32)
    w2p[:, :N_CLASSES] = W2
    in_maps = []
    for c in range(NCORES):
        xs = np.zeros((NSHP, F_IN), np.float32)
        xs[:NSH] = x[c * NSH : (c + 1) * NSH]
        in_maps.append(
            {
                "xT_s": np.ascontiguousarray(xs.T).astype(bf16),
                "w1": W1.astype(bf16),
                "w2": w2p.astype(bf16),
                "dinv_c": meta["dinv_cs"][c],
                "idx_w": meta["idx_ws"][c],
                "dst_w": meta["dst_ws"][c],
                "iota_r": meta["iota"],
            }
        )
    outs = st["runner"](in_maps)
    return np.concatenate([outs[c][0][:NSH] for c in range(NCORES)], axis=0)
